# revision 1
# baseline (speedup 1.0000x reference)
"""Trainium2 Bass kernel for nn_AffineLayer (topk_masking):
out[b, f] = max_p(x[b] . ww[f, p]) * scale[f] + bias[f]

Shapes (hardcoded per problem spec):
  x     (2048, 1, 8, 8)  -> xf (2048, 64)
  ww    (1024, 64, 1, 8, 8) -> wwf (1024, 64, 64)   (f, p, i)
  scale (1, 1024), bias (1, 1024)
  out   (2048, 1024)

Sharding: f tensor-parallel over 8 cores (f_shard = 128 per core), x replicated.

Per-core device layout (f on partitions):
  lhsT (stationary) = wT[:, p, :] : (i=64, f=128)  per p-plane
  rhs  (moving)     = xT[:, bchunk]: (i=64, b=512)
  psum out          = (f=128, b=512) per p-plane, 1 PSUM bank

The 64-way max over p is the bottleneck: every score must leave PSUM through
one of the only two PSUM-capable engines (DVE and ACT, both 1 elem/cycle/lane
for fp32). p-plane groups are split between:
  - DVE: running tensor_tensor(max) straight from PSUM into a 4-slot fp32 acc
  - ACT: activation(Copy) PSUM -> SBUF staging (cast to fp16: same 16-bit
    2x fold speed as bf16, 8x the mantissa precision), folded into 16-bit
    accumulators by DVE tensor_tensor at 2x packed rate.
Final per-chunk: fold acc slots, combine paths, apply scale/bias via one
tensor_scalar with per-partition (f) scalars, DMA out as (128f, 2048b).
Host reassembles and transposes to (2048, 1024).
"""

import os
import sys

if "/opt/trn_rl_repo" not in sys.path:
    sys.path.insert(0, "/opt/trn_rl_repo")

import numpy as np

import concourse.bass as bass
import concourse.mybir as mybir
from concourse.tile import TileContext
from concourse.bass_utils import run_bass_kernel_spmd

# Problem dims (hardcoded)
B, FDIM, P, IDIM = 2048, 1024, 64, 64
N_CORES = 8
F_SH = FDIM // N_CORES  # 128
BCH = 512  # b-chunk size (PSUM bank = 512 fp32)
NJ = B // BCH  # 4
GQ = int(os.environ.get("KGQ", "2"))  # p-planes per group (= PSUM banks)
NG = P // GQ  # groups
PSUM_BUFS = 8 // GQ

# ---- Tunables ----------------------------------------------------------
# Per-group drain assignment, length NG. "D" = DVE direct TT-max from PSUM;
# "V" = ACT copy -> staged, folded by DVE; "G" = ACT copy -> staged, folded
# by GPSIMD.
ASSIGN = os.environ.get("KASSIGN", "VVDVVVDVVVDVVVDVVVDVVVDVVVDVVVDV")
STAGE_BF16 = os.environ.get("KSTAGE_BF16", "1") == "1"
# Staging dtype: fp16 matches bf16's 2x DVE fold speed (both 16-bit) but has
# 10 mantissa bits vs 7 — scores (|s| < ~70) sit far inside fp16 range.
STAGE_DT_NAME = os.environ.get("KSTAGE_DT", "float16" if STAGE_BF16 else "float32")
# Matmul input dtype: float32r streams 1 row/cycle on the PE (vs 4 for fp32,
# which decomposes into 2 half-speed passes); same 4-byte layout as fp32.
MM_DT_NAME = os.environ.get("KMM_DT", "float32r")
# Unified accumulator: direct-drained groups also max into the bf16 staged
# acc (drops the separate fp32 acc + its tail folds; whole output ~bf16).
UNIFIED = os.environ.get("KUNIFIED", "0") == "1"
NWCH = int(os.environ.get("KNWCH", "16"))
REPS = int(os.environ.get("KREPS", "0"))  # >0: wrap body in a For_i repeat loop (bench only)
STAGE_BUFS = int(os.environ.get("KSTAGE_BUFS", "8"))
XT_CHUNKED = os.environ.get("KXT_CHUNKED", "1") == "1"
DQUAD = os.environ.get("KDQUAD", "0") == "1"  # D-groups drain as 4-bank quads
JINT = os.environ.get("KJINT", "0") == "1"  # interleave all b-chunks per p-position
# ------------------------------------------------------------------------

F32 = mybir.dt.float32
BF16 = mybir.dt.bfloat16
STAGE_DT = getattr(mybir.dt, STAGE_DT_NAME)
MM_DT = getattr(mybir.dt, MM_DT_NAME)
MX = mybir.AluOpType.max


def split_multiwaits(nc):
    """This walrus build allows at most ONE sem wait per instruction.
    Tile's wait assignment can emit several; hoist extras onto inserted
    sequencer nops immediately before the over-subscribed instruction
    (same engine, program order preserved => identical semantics)."""
    wid = 0
    for f in nc.m.functions:
        for bb in f.blocks:
            il = bb.instructions
            i = 0
            while i < len(il):
                ins = il[i]
                si = getattr(ins, "sync_info", None)
                if si is not None and si.on_wait and len(si.on_wait) > 1:
                    waits = list(si.on_wait)
                    si.on_wait = waits[-1:]
                    carriers = []
                    for w in waits[:-1]:
                        wid += 1
                        carriers.append(
                            mybir.InstNoOp(
                                name=f"WSPLIT-{wid}",
                                engine=ins.engine,
                                sync_info=mybir.SyncInfo(on_wait=[w], on_update=[]),
                            )
                        )
                    il[i:i] = carriers
                    i += len(carriers)
                i += 1


def build_nc_jint(assign=None, fixup=True, affine=True):
    """b-chunk-interleaved variant: iterate p-positions outer, all NJ b-chunks
    inner. Staged tiles hold one position x all chunks (NJ*GQ planes), folded
    by one DVE TT; accumulators span all chunks so the tails and the output
    DMA are whole-row ops."""
    assign = (assign or ASSIGN).split(";")[0]
    assert len(assign) in (16, NG) and set(assign) <= set("DV")
    if len(assign) != NG:
        assign = "".join(c * (NG // 16) for c in assign)
    last_d = assign.rfind("D")
    last_v = assign.rfind("V")

    nc = bass.Bass()
    xt_d = nc.dram_tensor("xt", [IDIM, B], MM_DT, kind="ExternalInput")
    wt_d = nc.dram_tensor("wt", [IDIM, P, F_SH], MM_DT, kind="ExternalInput")
    sc_d = nc.dram_tensor("scale", [F_SH, 1], F32, kind="ExternalInput")
    bi_d = nc.dram_tensor("bias", [F_SH, 1], F32, kind="ExternalInput")
    y_d = nc.dram_tensor("y", [F_SH, B], F32, kind="ExternalOutput")

    PW = P // NWCH

    with TileContext(nc) as tc:
        with (
            tc.tile_pool(name="const", bufs=1) as const,
            tc.tile_pool(name="psum", bufs=PSUM_BUFS, space="PSUM") as psum,
            tc.tile_pool(name="accs", bufs=2) as accs,
            tc.tile_pool(name="stage", bufs=STAGE_BUFS) as stage,
            tc.tile_pool(name="outs", bufs=2) as outs,
        ):
            xt = const.tile([IDIM, B], MM_DT)
            nc.sync.dma_start(out=xt[:, 0:BCH], in_=xt_d[:, 0:BCH])
            wchunks = [
                const.tile([IDIM, PW, F_SH], MM_DT, name=f"wt{c}") for c in range(NWCH)
            ]
            nc.sync.dma_start(out=wchunks[0][:], in_=wt_d[:, 0:PW, :])
            for c in range(1, NJ):
                nc.sync.dma_start(
                    out=xt[:, c * BCH : (c + 1) * BCH],
                    in_=xt_d[:, c * BCH : (c + 1) * BCH],
                )
            for c in range(1, NWCH):
                nc.sync.dma_start(
                    out=wchunks[c][:], in_=wt_d[:, c * PW : (c + 1) * PW, :]
                )
            sc = const.tile([F_SH, 1], F32)
            nc.sync.dma_start(out=sc[:], in_=sc_d[:])
            bi = const.tile([F_SH, 1], F32)
            nc.sync.dma_start(out=bi[:], in_=bi_d[:])
            warm = const.tile([F_SH, 2], F32)
            nc.vector.memset(warm[:], 0.0)
            nc.scalar.activation(
                out=warm[:, 1:2], in_=warm[:, 0:1],
                func=mybir.ActivationFunctionType.Copy,
            )

            import contextlib

            loop_cm = (
                tc.For_i(0, REPS, 1, hint_engines=(mybir.EngineType.PE,))
                if REPS > 0
                else contextlib.nullcontext()
            )
            with loop_cm:
                acc_d = accs.tile([F_SH, NJ, GQ, BCH], F32, tag="acc_d")
                acc_v = accs.tile([F_SH, NJ, GQ, BCH], STAGE_DT, tag="acc_v")
                n_d = n_v = 0

                def fold_gq(acc):
                    w = GQ
                    while w > 1:
                        h = w // 2
                        nc.vector.tensor_max(
                            acc[:, :, 0:h, :], acc[:, :, 0:h, :], acc[:, :, h:w, :]
                        )
                        w = h

                for g in range(NG):
                    if assign[g] == "D":
                        for j in range(NJ):
                            pt = psum.tile([F_SH, GQ, BCH], F32, tag="ps")
                            for q in range(GQ):
                                p = GQ * g + q
                                nc.tensor.matmul(
                                    pt[:, q, :],
                                    wchunks[p // PW][:, p % PW, :],
                                    xt[:, j * BCH : (j + 1) * BCH],
                                    start=True,
                                    stop=True,
                                )
                            dst = acc_d[:, j]
                            if n_d == 0:
                                nc.vector.tensor_copy(out=dst, in_=pt[:])
                            else:
                                nc.vector.tensor_max(dst, pt[:], dst)
                        n_d += 1
                        if g == last_d and last_d > last_v:
                            fold_gq(acc_d)
                    else:
                        st = stage.tile([F_SH, NJ, GQ, BCH], STAGE_DT, tag="st")
                        for j in range(NJ):
                            pt = psum.tile([F_SH, GQ, BCH], F32, tag="ps")
                            for q in range(GQ):
                                p = GQ * g + q
                                nc.tensor.matmul(
                                    pt[:, q, :],
                                    wchunks[p // PW][:, p % PW, :],
                                    xt[:, j * BCH : (j + 1) * BCH],
                                    start=True,
                                    stop=True,
                                )
                            nc.scalar.activation(
                                out=st[:, j],
                                in_=pt[:],
                                func=mybir.ActivationFunctionType.Copy,
                            )
                        if n_v == 0:
                            nc.vector.tensor_copy(out=acc_v[:], in_=st[:])
                        else:
                            nc.vector.tensor_max(acc_v[:], st[:], acc_v[:])
                        n_v += 1
                        if g == last_v and last_v > last_d:
                            fold_gq(acc_v)

                # ---- tails: whole-row ops across all chunks ------------
                if n_v and last_v < last_d:
                    fold_gq(acc_v)
                if n_d and last_d < last_v:
                    fold_gq(acc_d)
                staged = acc_v[:, :, 0, :] if n_v else None  # (F_SH, NJ, BCH)
                direct = acc_d[:, :, 0, :] if n_d else None
                outt = outs.tile([F_SH, NJ, BCH], F32, tag="outt")
                if direct is not None and staged is not None:
                    nc.vector.tensor_max(outt[:], direct, staged)
                    src = outt[:]
                elif direct is not None:
                    src = direct
                else:
                    src = staged
                if affine:
                    nc.vector.tensor_scalar(
                        out=outt[:],
                        in0=src,
                        scalar1=sc[:],
                        scalar2=bi[:],
                        op0=mybir.AluOpType.mult,
                        op1=mybir.AluOpType.add,
                    )
                    src = outt[:]
                elif src is not outt[:] and src.dtype != F32:
                    nc.vector.tensor_copy(out=outt[:], in_=src)
                    src = outt[:]
                nc.sync.dma_start(out=y_d[:], in_=src)

    if fixup:
        split_multiwaits(nc)
    return nc



def build_nc(assign=None, fixup=True, affine=True):
    if JINT:
        return build_nc_jint(assign=assign, fixup=fixup, affine=affine)
    assign = assign or ASSIGN
    pats = assign.split(";")
    if len(pats) == 1:
        pats = pats * NJ
    assert len(pats) == NJ
    expanded = []
    for p_ in pats:
        assert len(p_) in (16, NG) and set(p_) <= set("DV")
        if len(p_) != NG:
            p_ = "".join(c * (NG // 16) for c in p_)
        expanded.append(p_)
    pats = expanded

    nc = bass.Bass()
    xt_d = nc.dram_tensor("xt", [IDIM, B], MM_DT, kind="ExternalInput")
    wt_d = nc.dram_tensor("wt", [IDIM, P, F_SH], MM_DT, kind="ExternalInput")
    sc_d = nc.dram_tensor("scale", [F_SH, 1], F32, kind="ExternalInput")
    bi_d = nc.dram_tensor("bias", [F_SH, 1], F32, kind="ExternalInput")
    y_d = nc.dram_tensor("y", [F_SH, B], F32, kind="ExternalOutput")

    PW = P // NWCH  # p-planes per weight chunk
    VS = 2 * GQ  # staged-pair slot count (2 groups per staged tile)

    with TileContext(nc) as tc:
        with (
            tc.tile_pool(name="const", bufs=1) as const,
            tc.tile_pool(name="psum", bufs=PSUM_BUFS, space="PSUM") as psum,
            tc.tile_pool(
                name="accs", bufs=int(os.environ.get("KACC_BUFS", "3"))
            ) as accs,
            tc.tile_pool(name="stage", bufs=STAGE_BUFS) as stage,
            tc.tile_pool(
                name="outs", bufs=int(os.environ.get("KOUT_BUFS", "3"))
            ) as outs,
        ):
            # input loads: first-needed chunks first so group 0 starts ASAP
            xt = const.tile([IDIM, B], MM_DT)
            wchunks = [
                const.tile([IDIM, PW, F_SH], MM_DT, name=f"wt{c}") for c in range(NWCH)
            ]
            nc.sync.dma_start(out=xt[:, 0:BCH], in_=xt_d[:, 0:BCH])
            nc.sync.dma_start(out=wchunks[0][:], in_=wt_d[:, 0:PW, :])
            nc.sync.dma_start(out=wchunks[1][:], in_=wt_d[:, PW : 2 * PW, :])
            for c in range(2, NWCH):
                nc.sync.dma_start(
                    out=wchunks[c][:], in_=wt_d[:, c * PW : (c + 1) * PW, :]
                )
            for c in range(1, NJ):
                nc.sync.dma_start(
                    out=xt[:, c * BCH : (c + 1) * BCH],
                    in_=xt_d[:, c * BCH : (c + 1) * BCH],
                )
            sc = const.tile([F_SH, 1], F32)
            nc.sync.dma_start(out=sc[:], in_=sc_d[:])
            bi = const.tile([F_SH, 1], F32)
            nc.sync.dma_start(out=bi[:], in_=bi_d[:])
            warm = const.tile([F_SH, 2], F32)
            nc.vector.memset(warm[:], 0.0)
            nc.scalar.activation(
                out=warm[:, 1:2], in_=warm[:, 0:1],
                func=mybir.ActivationFunctionType.Copy,
            )

            import contextlib

            loop_cm = (
                tc.For_i(0, REPS, 1, hint_engines=(mybir.EngineType.PE,))
                if REPS > 0
                else contextlib.nullcontext()
            )
            with loop_cm:
              for j in range(NJ):
                assign_j = pats[j]
                last_d = assign_j.rfind("D")
                rhs = xt[:, j * BCH : (j + 1) * BCH]
                DS = 4 if DQUAD else GQ
                acc_d = accs.tile([F_SH, DS, BCH], F32, tag="acc_d")
                acc_v = accs.tile([F_SH, VS, BCH], STAGE_DT, tag="acc_v")
                n_d = n_v = 0
                half = 0  # staged-pair fill state
                st = None

                def flush_pair(full):
                    nonlocal n_v, st
                    if full:
                        src = st[:].rearrange("p a g b -> p (a g) b")
                        dst = acc_v[:]
                    else:
                        src = st[:, 0]
                        dst = acc_v[:, 0:GQ, :]
                    if n_v == 0:
                        nc.vector.tensor_copy(out=dst, in_=src)
                    else:
                        nc.vector.tensor_max(dst, src, dst)
                    n_v += 1
                    st = None

                # build token schedule: D-pairs become 4-bank quads in DQUAD mode
                tokens = []
                g = 0
                while g < NG:
                    if (
                        DQUAD
                        and assign_j[g] == "D"
                    ):
                        assert g + 1 < NG and assign_j[g + 1] == "D", (
                            "KDQUAD=1 requires D groups in adjacent pairs"
                        )
                        tokens.append(("D", g, 2 * GQ))
                        g += 2
                    else:
                        tokens.append((assign_j[g], g, GQ))
                        g += 1
                n_dtok = sum(1 for t in tokens if t[0] == "D")
                dtok_i = 0
                for kind, g0, nplanes in tokens:
                    if kind == "D" and DQUAD:
                        pt = psum.tile([F_SH, 4, BCH], F32, tag="psd", bufs=1, name="ptd")
                    else:
                        pt = psum.tile(
                            [F_SH, GQ, BCH],
                            F32,
                            tag="ps",
                            bufs=2 if DQUAD else PSUM_BUFS,
                            name="ptv",
                        )
                    for q in range(nplanes):
                        p = GQ * g0 + q
                        nc.tensor.matmul(
                            pt[:, q, :],
                            wchunks[p // PW][:, p % PW, :],
                            rhs,
                            start=True,
                            stop=True,
                        )
                    if kind == "D":
                        dst = acc_d[:] if nplanes == DS else acc_d[:, 0:nplanes, :]
                        if n_d == 0:
                            assert nplanes == DS, "first D token must fill acc_d"
                            nc.vector.tensor_copy(out=dst, in_=pt[:])
                        else:
                            nc.vector.tensor_max(dst, pt[:], dst)
                        n_d += 1
                        dtok_i += 1
                        if dtok_i == n_dtok:
                            w = DS
                            while w > 1:
                                h = w // 2
                                nc.vector.tensor_max(
                                    acc_d[:, 0:h, :],
                                    acc_d[:, 0:h, :],
                                    acc_d[:, h:w, :],
                                )
                                w = h
                    else:
                        if st is None:
                            st = stage.tile([F_SH, 2, GQ, BCH], STAGE_DT, tag="st")
                        nc.scalar.activation(
                            out=st[:, half],
                            in_=pt[:],
                            func=mybir.ActivationFunctionType.Copy,
                        )
                        half ^= 1
                        if half == 0:
                            flush_pair(full=True)
                if half == 1:
                    flush_pair(full=False)

                # ---- tails ------------------------------------------------
                staged = None
                if n_v:
                    w = VS
                    while w > 1:
                        h = w // 2
                        nc.vector.tensor_max(
                            acc_v[:, 0:h, :], acc_v[:, 0:h, :], acc_v[:, h:w, :]
                        )
                        w = h
                    staged = acc_v[:, 0, :]
                direct = acc_d[:, 0, :] if n_d else None

                outt = outs.tile([F_SH, BCH], F32, tag="outt")
                if direct is not None and staged is not None:
                    nc.vector.tensor_max(outt[:], direct, staged)  # mixed dtype OK
                    src = outt[:]
                elif direct is not None:
                    src = direct
                else:
                    src = staged
                if affine:
                    nc.vector.tensor_scalar(
                        out=outt[:],
                        in0=src,
                        scalar1=sc[:],
                        scalar2=bi[:],
                        op0=mybir.AluOpType.mult,
                        op1=mybir.AluOpType.add,
                    )
                    src = outt[:]
                elif src is not outt[:] and src.dtype != F32:
                    nc.vector.tensor_copy(out=outt[:], in_=src)
                    src = outt[:]
                nc.sync.dma_start(out=y_d[:, j * BCH : (j + 1) * BCH], in_=src)

    if fixup:
        split_multiwaits(nc)
    return nc


_CACHED_NC = None


def _get_nc():
    global _CACHED_NC
    if _CACHED_NC is None:
        _CACHED_NC = build_nc()
    return _CACHED_NC


def make_in_maps(x, ww, scale, bias):
    x = np.asarray(x)
    ww = np.asarray(ww)
    scale = np.asarray(scale)
    bias = np.asarray(bias)

    xf = np.ascontiguousarray(x.reshape(B, IDIM).T).astype(np.float32)  # (64, 2048)
    wwf = ww.reshape(FDIM, P, IDIM)
    sc = scale.reshape(FDIM).astype(np.float32)
    bi = bias.reshape(FDIM).astype(np.float32)

    in_maps = []
    for k in range(N_CORES):
        wk = wwf[k * F_SH : (k + 1) * F_SH]  # (128, 64, 64) = (f, p, i)
        wt = np.ascontiguousarray(wk.transpose(2, 1, 0)).astype(np.float32)  # (i,p,f)
        in_maps.append(
            {
                "xt": xf,
                "wt": wt,
                "scale": np.ascontiguousarray(
                    sc[k * F_SH : (k + 1) * F_SH].reshape(F_SH, 1)
                ),
                "bias": np.ascontiguousarray(
                    bi[k * F_SH : (k + 1) * F_SH].reshape(F_SH, 1)
                ),
            }
        )
    return in_maps


def kernel(x, ww, scale, bias):
    in_maps = make_in_maps(x, ww, scale, bias)
    trivial_affine = bool(
        np.all(np.asarray(scale) == 1.0) and np.all(np.asarray(bias) == 0.0)
    )
    nc = build_nc(affine=not trivial_affine)
    res = run_bass_kernel_spmd(nc, in_maps, list(range(N_CORES)))
    out = np.empty((FDIM, B), dtype=np.float32)
    for k in range(N_CORES):
        out[k * F_SH : (k + 1) * F_SH] = res.results[k]["y"]
    return np.ascontiguousarray(out.T)



# revision 4
# speedup vs baseline: 1.1273x; 1.1273x over previous
"""Trainium2 Bass kernel for nn_AffineLayer (topk_masking):
out[b, f] = max_p(x[b] . ww[f, p]) * scale[f] + bias[f]

Shapes (hardcoded per problem spec):
  x     (2048, 1, 8, 8)  -> xf (2048, 64)
  ww    (1024, 64, 1, 8, 8) -> wwf (1024, 64, 64)   (f, p, i)
  scale (1, 1024), bias (1, 1024)
  out   (2048, 1024)

Sharding: f tensor-parallel over 8 cores (f_shard = 128 per core), x replicated.

Per-core device layout (f on partitions):
  lhsT (stationary) = wT[:, p, :] : (i=64, f=128)  per p-plane
  rhs  (moving)     = xT[:, bchunk]: (i=64, b=512)
  psum out          = (f=128, b=512) per p-plane, 1 PSUM bank

The 64-way max over p is the bottleneck: every score must leave PSUM through
one of the only two PSUM-capable engines (DVE and ACT, both 1 elem/cycle/lane
for fp32). p-plane groups are split between:
  - DVE: running tensor_tensor(max) straight from PSUM into a 4-slot fp32 acc
  - ACT: activation(Copy) PSUM -> SBUF staging (cast to fp16: same 16-bit
    2x fold speed as bf16, 8x the mantissa precision), folded into 16-bit
    accumulators by DVE tensor_tensor at 2x packed rate.
Final per-chunk: fold acc slots, combine paths, apply scale/bias via one
tensor_scalar with per-partition (f) scalars, DMA out as (128f, 2048b).
Host reassembles and transposes to (2048, 1024).
"""

import os
import sys

if "/opt/trn_rl_repo" not in sys.path:
    sys.path.insert(0, "/opt/trn_rl_repo")

import numpy as np

import concourse.bass as bass
import concourse.mybir as mybir
from concourse.tile import TileContext
from concourse.bass_utils import run_bass_kernel_spmd

# Problem dims (hardcoded)
B, FDIM, P, IDIM = 2048, 1024, 64, 64
N_CORES = 8
F_SH = FDIM // N_CORES  # 128
BCH = 512  # b-chunk size (PSUM bank = 512 fp32)
NJ = B // BCH  # 4
GQ = int(os.environ.get("KGQ", "2"))  # p-planes per group (= PSUM banks)
NG = P // GQ  # groups
PSUM_BUFS = 8 // GQ

# ---- Tunables ----------------------------------------------------------
# Per-group drain assignment, length NG. "D" = DVE direct TT-max from PSUM;
# "V" = ACT copy -> staged, folded by DVE; "G" = ACT copy -> staged, folded
# by GPSIMD.
ASSIGN = os.environ.get("KASSIGN", "VVDVVVDVVVDVVVDVVVDVVVDVVVDVVVDV")
STAGE_BF16 = os.environ.get("KSTAGE_BF16", "1") == "1"
# Staging dtype: fp16 matches bf16's 2x DVE fold speed (both 16-bit) but has
# 10 mantissa bits vs 7 — scores (|s| < ~70) sit far inside fp16 range.
STAGE_DT_NAME = os.environ.get("KSTAGE_DT", "float16" if STAGE_BF16 else "float32")
# Matmul input dtype: float32r streams 1 row/cycle on the PE (vs 4 for fp32,
# which decomposes into 2 half-speed passes); same 4-byte layout as fp32.
MM_DT_NAME = os.environ.get("KMM_DT", "float32r")
# Unified accumulator: direct-drained groups also max into the bf16 staged
# acc (drops the separate fp32 acc + its tail folds; whole output ~bf16).
UNIFIED = os.environ.get("KUNIFIED", "0") == "1"
NWCH = int(os.environ.get("KNWCH", "16"))
REPS = int(os.environ.get("KREPS", "0"))  # >0: wrap body in a For_i repeat loop (bench only)
STAGE_BUFS = int(os.environ.get("KSTAGE_BUFS", "8"))
XT_CHUNKED = os.environ.get("KXT_CHUNKED", "1") == "1"
DQUAD = os.environ.get("KDQUAD", "0") == "1"  # D-groups drain as 4-bank quads
JINT = os.environ.get("KJINT", "0") == "1"  # interleave all b-chunks per p-position
# ------------------------------------------------------------------------

F32 = mybir.dt.float32
BF16 = mybir.dt.bfloat16
STAGE_DT = getattr(mybir.dt, STAGE_DT_NAME)
MM_DT = getattr(mybir.dt, MM_DT_NAME)
MX = mybir.AluOpType.max


def split_multiwaits(nc):
    """This walrus build allows at most ONE sem wait per instruction.
    Tile's wait assignment can emit several; hoist extras onto inserted
    sequencer nops immediately before the over-subscribed instruction
    (same engine, program order preserved => identical semantics)."""
    wid = 0
    for f in nc.m.functions:
        for bb in f.blocks:
            il = bb.instructions
            i = 0
            while i < len(il):
                ins = il[i]
                si = getattr(ins, "sync_info", None)
                if si is not None and si.on_wait and len(si.on_wait) > 1:
                    waits = list(si.on_wait)
                    si.on_wait = waits[-1:]
                    carriers = []
                    for w in waits[:-1]:
                        wid += 1
                        carriers.append(
                            mybir.InstNoOp(
                                name=f"WSPLIT-{wid}",
                                engine=ins.engine,
                                sync_info=mybir.SyncInfo(on_wait=[w], on_update=[]),
                            )
                        )
                    il[i:i] = carriers
                    i += len(carriers)
                i += 1


def build_nc_jint(assign=None, fixup=True, affine=True):
    """b-chunk-interleaved variant: iterate p-positions outer, all NJ b-chunks
    inner. Staged tiles hold one position x all chunks (NJ*GQ planes), folded
    by one DVE TT; accumulators span all chunks so the tails and the output
    DMA are whole-row ops."""
    assign = (assign or ASSIGN).split(";")[0]
    assert len(assign) in (16, NG) and set(assign) <= set("DV")
    if len(assign) != NG:
        assign = "".join(c * (NG // 16) for c in assign)
    last_d = assign.rfind("D")
    last_v = assign.rfind("V")

    nc = bass.Bass()
    xt_d = nc.dram_tensor("xt", [IDIM, B], MM_DT, kind="ExternalInput")
    wt_d = nc.dram_tensor("wt", [IDIM, P, F_SH], MM_DT, kind="ExternalInput")
    sc_d = nc.dram_tensor("scale", [F_SH, 1], F32, kind="ExternalInput")
    bi_d = nc.dram_tensor("bias", [F_SH, 1], F32, kind="ExternalInput")
    y_d = nc.dram_tensor("y", [F_SH, B], F32, kind="ExternalOutput")

    PW = P // NWCH

    with TileContext(nc) as tc:
        with (
            tc.tile_pool(name="const", bufs=1) as const,
            tc.tile_pool(name="psum", bufs=PSUM_BUFS, space="PSUM") as psum,
            tc.tile_pool(name="accs", bufs=2) as accs,
            tc.tile_pool(name="stage", bufs=STAGE_BUFS) as stage,
            tc.tile_pool(name="outs", bufs=2) as outs,
        ):
            xt = const.tile([IDIM, B], MM_DT)
            nc.sync.dma_start(out=xt[:, 0:BCH], in_=xt_d[:, 0:BCH])
            wchunks = [
                const.tile([IDIM, PW, F_SH], MM_DT, name=f"wt{c}") for c in range(NWCH)
            ]
            nc.sync.dma_start(out=wchunks[0][:], in_=wt_d[:, 0:PW, :])
            for c in range(1, NJ):
                nc.sync.dma_start(
                    out=xt[:, c * BCH : (c + 1) * BCH],
                    in_=xt_d[:, c * BCH : (c + 1) * BCH],
                )
            for c in range(1, NWCH):
                nc.sync.dma_start(
                    out=wchunks[c][:], in_=wt_d[:, c * PW : (c + 1) * PW, :]
                )
            sc = const.tile([F_SH, 1], F32)
            nc.sync.dma_start(out=sc[:], in_=sc_d[:])
            bi = const.tile([F_SH, 1], F32)
            nc.sync.dma_start(out=bi[:], in_=bi_d[:])
            warm = const.tile([F_SH, 2], F32)
            nc.vector.memset(warm[:], 0.0)
            nc.scalar.activation(
                out=warm[:, 1:2], in_=warm[:, 0:1],
                func=mybir.ActivationFunctionType.Copy,
            )

            import contextlib

            loop_cm = (
                tc.For_i(0, REPS, 1, hint_engines=(mybir.EngineType.PE,))
                if REPS > 0
                else contextlib.nullcontext()
            )
            with loop_cm:
                acc_d = accs.tile([F_SH, NJ, GQ, BCH], F32, tag="acc_d")
                acc_v = accs.tile([F_SH, NJ, GQ, BCH], STAGE_DT, tag="acc_v")
                n_d = n_v = 0

                def fold_gq(acc):
                    w = GQ
                    while w > 1:
                        h = w // 2
                        nc.vector.tensor_max(
                            acc[:, :, 0:h, :], acc[:, :, 0:h, :], acc[:, :, h:w, :]
                        )
                        w = h

                for g in range(NG):
                    if assign[g] == "D":
                        for j in range(NJ):
                            pt = psum.tile([F_SH, GQ, BCH], F32, tag="ps")
                            for q in range(GQ):
                                p = GQ * g + q
                                nc.tensor.matmul(
                                    pt[:, q, :],
                                    wchunks[p // PW][:, p % PW, :],
                                    xt[:, j * BCH : (j + 1) * BCH],
                                    start=True,
                                    stop=True,
                                )
                            dst = acc_d[:, j]
                            if n_d == 0:
                                nc.vector.tensor_copy(out=dst, in_=pt[:])
                            else:
                                nc.vector.tensor_max(dst, pt[:], dst)
                        n_d += 1
                        if g == last_d and last_d > last_v:
                            fold_gq(acc_d)
                    else:
                        st = stage.tile([F_SH, NJ, GQ, BCH], STAGE_DT, tag="st")
                        for j in range(NJ):
                            pt = psum.tile([F_SH, GQ, BCH], F32, tag="ps")
                            for q in range(GQ):
                                p = GQ * g + q
                                nc.tensor.matmul(
                                    pt[:, q, :],
                                    wchunks[p // PW][:, p % PW, :],
                                    xt[:, j * BCH : (j + 1) * BCH],
                                    start=True,
                                    stop=True,
                                )
                            nc.scalar.activation(
                                out=st[:, j],
                                in_=pt[:],
                                func=mybir.ActivationFunctionType.Copy,
                            )
                        if n_v == 0:
                            nc.vector.tensor_copy(out=acc_v[:], in_=st[:])
                        else:
                            nc.vector.tensor_max(acc_v[:], st[:], acc_v[:])
                        n_v += 1
                        if g == last_v and last_v > last_d:
                            fold_gq(acc_v)

                # ---- tails: whole-row ops across all chunks ------------
                if n_v and last_v < last_d:
                    fold_gq(acc_v)
                if n_d and last_d < last_v:
                    fold_gq(acc_d)
                staged = acc_v[:, :, 0, :] if n_v else None  # (F_SH, NJ, BCH)
                direct = acc_d[:, :, 0, :] if n_d else None
                outt = outs.tile([F_SH, NJ, BCH], F32, tag="outt")
                if direct is not None and staged is not None:
                    nc.vector.tensor_max(outt[:], direct, staged)
                    src = outt[:]
                elif direct is not None:
                    src = direct
                else:
                    src = staged
                if affine:
                    nc.vector.tensor_scalar(
                        out=outt[:],
                        in0=src,
                        scalar1=sc[:],
                        scalar2=bi[:],
                        op0=mybir.AluOpType.mult,
                        op1=mybir.AluOpType.add,
                    )
                    src = outt[:]
                elif src is not outt[:] and src.dtype != F32:
                    nc.vector.tensor_copy(out=outt[:], in_=src)
                    src = outt[:]
                nc.sync.dma_start(out=y_d[:], in_=src)

    if fixup:
        split_multiwaits(nc)
    return nc



def build_nc(assign=None, fixup=True, affine=True):
    if os.environ.get("KV", "2") == "2":
        return build_nc_v2(fixup=fixup, affine=affine)
    if JINT:
        return build_nc_jint(assign=assign, fixup=fixup, affine=affine)
    assign = assign or ASSIGN
    pats = assign.split(";")
    if len(pats) == 1:
        pats = pats * NJ
    assert len(pats) == NJ
    expanded = []
    for p_ in pats:
        assert len(p_) in (16, NG) and set(p_) <= set("DV")
        if len(p_) != NG:
            p_ = "".join(c * (NG // 16) for c in p_)
        expanded.append(p_)
    pats = expanded

    nc = bass.Bass()
    xt_d = nc.dram_tensor("xt", [IDIM, B], MM_DT, kind="ExternalInput")
    wt_d = nc.dram_tensor("wt", [IDIM, P, F_SH], MM_DT, kind="ExternalInput")
    sc_d = nc.dram_tensor("scale", [F_SH, 1], F32, kind="ExternalInput")
    bi_d = nc.dram_tensor("bias", [F_SH, 1], F32, kind="ExternalInput")
    y_d = nc.dram_tensor("y", [F_SH, B], F32, kind="ExternalOutput")

    PW = P // NWCH  # p-planes per weight chunk
    VS = 2 * GQ  # staged-pair slot count (2 groups per staged tile)

    with TileContext(nc) as tc:
        with (
            tc.tile_pool(name="const", bufs=1) as const,
            tc.tile_pool(name="psum", bufs=PSUM_BUFS, space="PSUM") as psum,
            tc.tile_pool(
                name="accs", bufs=int(os.environ.get("KACC_BUFS", "3"))
            ) as accs,
            tc.tile_pool(name="stage", bufs=STAGE_BUFS) as stage,
            tc.tile_pool(
                name="outs", bufs=int(os.environ.get("KOUT_BUFS", "3"))
            ) as outs,
        ):
            # input loads: first-needed chunks first so group 0 starts ASAP
            xt = const.tile([IDIM, B], MM_DT)
            wchunks = [
                const.tile([IDIM, PW, F_SH], MM_DT, name=f"wt{c}") for c in range(NWCH)
            ]
            nc.sync.dma_start(out=xt[:, 0:BCH], in_=xt_d[:, 0:BCH])
            nc.sync.dma_start(out=wchunks[0][:], in_=wt_d[:, 0:PW, :])
            nc.sync.dma_start(out=wchunks[1][:], in_=wt_d[:, PW : 2 * PW, :])
            for c in range(2, NWCH):
                nc.sync.dma_start(
                    out=wchunks[c][:], in_=wt_d[:, c * PW : (c + 1) * PW, :]
                )
            for c in range(1, NJ):
                nc.sync.dma_start(
                    out=xt[:, c * BCH : (c + 1) * BCH],
                    in_=xt_d[:, c * BCH : (c + 1) * BCH],
                )
            sc = const.tile([F_SH, 1], F32)
            nc.sync.dma_start(out=sc[:], in_=sc_d[:])
            bi = const.tile([F_SH, 1], F32)
            nc.sync.dma_start(out=bi[:], in_=bi_d[:])
            warm = const.tile([F_SH, 2], F32)
            nc.vector.memset(warm[:], 0.0)
            nc.scalar.activation(
                out=warm[:, 1:2], in_=warm[:, 0:1],
                func=mybir.ActivationFunctionType.Copy,
            )

            import contextlib

            loop_cm = (
                tc.For_i(0, REPS, 1, hint_engines=(mybir.EngineType.PE,))
                if REPS > 0
                else contextlib.nullcontext()
            )
            with loop_cm:
              for j in range(NJ):
                assign_j = pats[j]
                last_d = assign_j.rfind("D")
                rhs = xt[:, j * BCH : (j + 1) * BCH]
                DS = 4 if DQUAD else GQ
                acc_d = accs.tile([F_SH, DS, BCH], F32, tag="acc_d")
                acc_v = accs.tile([F_SH, VS, BCH], STAGE_DT, tag="acc_v")
                n_d = n_v = 0
                half = 0  # staged-pair fill state
                st = None

                def flush_pair(full):
                    nonlocal n_v, st
                    if full:
                        src = st[:].rearrange("p a g b -> p (a g) b")
                        dst = acc_v[:]
                    else:
                        src = st[:, 0]
                        dst = acc_v[:, 0:GQ, :]
                    if n_v == 0:
                        nc.vector.tensor_copy(out=dst, in_=src)
                    else:
                        nc.vector.tensor_max(dst, src, dst)
                    n_v += 1
                    st = None

                # build token schedule: D-pairs become 4-bank quads in DQUAD mode
                tokens = []
                g = 0
                while g < NG:
                    if (
                        DQUAD
                        and assign_j[g] == "D"
                    ):
                        assert g + 1 < NG and assign_j[g + 1] == "D", (
                            "KDQUAD=1 requires D groups in adjacent pairs"
                        )
                        tokens.append(("D", g, 2 * GQ))
                        g += 2
                    else:
                        tokens.append((assign_j[g], g, GQ))
                        g += 1
                n_dtok = sum(1 for t in tokens if t[0] == "D")
                dtok_i = 0
                for kind, g0, nplanes in tokens:
                    if kind == "D" and DQUAD:
                        pt = psum.tile([F_SH, 4, BCH], F32, tag="psd", bufs=1, name="ptd")
                    else:
                        pt = psum.tile(
                            [F_SH, GQ, BCH],
                            F32,
                            tag="ps",
                            bufs=2 if DQUAD else PSUM_BUFS,
                            name="ptv",
                        )
                    for q in range(nplanes):
                        p = GQ * g0 + q
                        nc.tensor.matmul(
                            pt[:, q, :],
                            wchunks[p // PW][:, p % PW, :],
                            rhs,
                            start=True,
                            stop=True,
                        )
                    if kind == "D":
                        dst = acc_d[:] if nplanes == DS else acc_d[:, 0:nplanes, :]
                        if n_d == 0:
                            assert nplanes == DS, "first D token must fill acc_d"
                            nc.vector.tensor_copy(out=dst, in_=pt[:])
                        else:
                            nc.vector.tensor_max(dst, pt[:], dst)
                        n_d += 1
                        dtok_i += 1
                        if dtok_i == n_dtok:
                            w = DS
                            while w > 1:
                                h = w // 2
                                nc.vector.tensor_max(
                                    acc_d[:, 0:h, :],
                                    acc_d[:, 0:h, :],
                                    acc_d[:, h:w, :],
                                )
                                w = h
                    else:
                        if st is None:
                            st = stage.tile([F_SH, 2, GQ, BCH], STAGE_DT, tag="st")
                        nc.scalar.activation(
                            out=st[:, half],
                            in_=pt[:],
                            func=mybir.ActivationFunctionType.Copy,
                        )
                        half ^= 1
                        if half == 0:
                            flush_pair(full=True)
                if half == 1:
                    flush_pair(full=False)

                # ---- tails ------------------------------------------------
                staged = None
                if n_v:
                    w = VS
                    while w > 1:
                        h = w // 2
                        nc.vector.tensor_max(
                            acc_v[:, 0:h, :], acc_v[:, 0:h, :], acc_v[:, h:w, :]
                        )
                        w = h
                    staged = acc_v[:, 0, :]
                direct = acc_d[:, 0, :] if n_d else None

                outt = outs.tile([F_SH, BCH], F32, tag="outt")
                if direct is not None and staged is not None:
                    nc.vector.tensor_max(outt[:], direct, staged)  # mixed dtype OK
                    src = outt[:]
                elif direct is not None:
                    src = direct
                else:
                    src = staged
                if affine:
                    nc.vector.tensor_scalar(
                        out=outt[:],
                        in0=src,
                        scalar1=sc[:],
                        scalar2=bi[:],
                        op0=mybir.AluOpType.mult,
                        op1=mybir.AluOpType.add,
                    )
                    src = outt[:]
                elif src is not outt[:] and src.dtype != F32:
                    nc.vector.tensor_copy(out=outt[:], in_=src)
                    src = outt[:]
                nc.sync.dma_start(out=y_d[:, j * BCH : (j + 1) * BCH], in_=src)

    if fixup:
        split_multiwaits(nc)
    return nc


def build_nc_v2(assign=None, fixup=True, affine=True):
    """v2: unified fp16 accumulator drain.

    Per b-chunk (BCH=512), 32 GQ=2 PSUM groups (4 in flight):
      - V-groups: ACT copies PSUM -> fp16. The first two fill the 4-slot
        acc directly (no TensorCopy init); later ones pair up in staged
        tiles folded into acc by DVE fp16 TTs at 2x.
      - D-groups: DVE tensor_tensor(max) PSUM(fp32) x acc(fp16) -> acc.
    Tail: two fp16 folds (4->2->1) with the last writing the y chunk;
    y is fp16 in DRAM, host upcasts. Affine (when non-trivial) is a 4x
    fp16 tensor_scalar on the y chunk.
    """
    assign = assign or os.environ.get(
        "KPAT", "VVDVVDVVDVVDVVDVVDVVDVVDVVDVVVVV"
    )
    assert len(assign) == 32 and set(assign) <= set("DV")
    assert assign[0] == "V" and assign[1] == "V", "first two groups must fill acc"

    nc = bass.Bass()
    xt_d = nc.dram_tensor("xt", [IDIM, B], MM_DT, kind="ExternalInput")
    wt_d = nc.dram_tensor("wt", [IDIM, P, F_SH], MM_DT, kind="ExternalInput")
    sc_d = nc.dram_tensor("scale", [F_SH, 1], F32, kind="ExternalInput")
    bi_d = nc.dram_tensor("bias", [F_SH, 1], F32, kind="ExternalInput")
    y_d = nc.dram_tensor("y", [F_SH, B], STAGE_DT, kind="ExternalOutput")

    PW = P // NWCH

    with TileContext(nc) as tc:
        with (
            tc.tile_pool(name="const", bufs=1) as const,
            tc.tile_pool(name="psum", bufs=4, space="PSUM") as psum,
            tc.tile_pool(name="accs", bufs=2) as accs,
            tc.tile_pool(name="stage", bufs=int(os.environ.get("KSTB", "6"))) as stage,
        ):
            xt = const.tile([IDIM, B], MM_DT)
            wchunks = [
                const.tile([IDIM, PW, F_SH], MM_DT, name=f"wt{c}") for c in range(NWCH)
            ]
            nc.sync.dma_start(out=xt[:, 0:BCH], in_=xt_d[:, 0:BCH])
            nc.sync.dma_start(out=wchunks[0][:], in_=wt_d[:, 0:PW, :])
            nc.sync.dma_start(out=wchunks[1][:], in_=wt_d[:, PW : 2 * PW, :])
            for c in range(2, NWCH):
                nc.sync.dma_start(
                    out=wchunks[c][:], in_=wt_d[:, c * PW : (c + 1) * PW, :]
                )
            for c in range(1, NJ):
                nc.sync.dma_start(
                    out=xt[:, c * BCH : (c + 1) * BCH],
                    in_=xt_d[:, c * BCH : (c + 1) * BCH],
                )
            sc = const.tile([F_SH, 1], F32)
            nc.sync.dma_start(out=sc[:], in_=sc_d[:])
            bi = const.tile([F_SH, 1], F32)
            nc.sync.dma_start(out=bi[:], in_=bi_d[:])
            y_sb = const.tile([F_SH, B], STAGE_DT, name="ysb")
            warm = const.tile([F_SH, 2], F32)
            nc.vector.memset(warm[:], 0.0)
            nc.scalar.activation(
                out=warm[:, 1:2], in_=warm[:, 0:1],
                func=mybir.ActivationFunctionType.Copy,
            )

            import contextlib

            loop_cm = (
                tc.For_i(0, REPS, 1, hint_engines=(mybir.EngineType.PE,))
                if REPS > 0
                else contextlib.nullcontext()
            )
            with loop_cm:
                for j in range(NJ):
                    rhs = xt[:, j * BCH : (j + 1) * BCH]
                    acc = accs.tile([F_SH, 2, GQ, BCH], STAGE_DT, tag="acc")
                    acc4 = acc[:].rearrange("p a g b -> p (a g) b")
                    nfill = 0  # V-groups that went straight into acc
                    pend = None  # half-filled staged pair tile
                    st = None

                    def mm_group(g):
                        pt = psum.tile([F_SH, GQ, BCH], F32, tag="ps")
                        for q in range(GQ):
                            p = GQ * g + q
                            nc.tensor.matmul(
                                pt[:, q, :],
                                wchunks[p // PW][:, p % PW, :],
                                rhs,
                                start=True,
                                stop=True,
                            )
                        return pt

                    for g in range(NG):
                        pt = mm_group(g)
                        if assign[g] == "V":
                            if nfill < 2:
                                nc.scalar.activation(
                                    out=acc[:, nfill],
                                    in_=pt[:],
                                    func=mybir.ActivationFunctionType.Copy,
                                )
                                nfill += 1
                            else:
                                if st is None:
                                    st = stage.tile(
                                        [F_SH, 2, GQ, BCH], STAGE_DT, tag="st"
                                    )
                                    half = 0
                                else:
                                    half = 1
                                nc.scalar.activation(
                                    out=st[:, half],
                                    in_=pt[:],
                                    func=mybir.ActivationFunctionType.Copy,
                                )
                                if half == 1:
                                    nc.vector.tensor_max(
                                        acc4,
                                        st[:].rearrange("p a g b -> p (a g) b"),
                                        acc4,
                                    )
                                    st = None
                        else:
                            # direct: PSUM fp32 x acc fp16 -> acc fp16 (1x)
                            nc.vector.tensor_max(acc[:, 0], pt[:], acc[:, 0])
                    if st is not None:  # odd V-group left staged
                        nc.vector.tensor_max(acc[:, 1], st[:, 0], acc[:, 1])
                        st = None

                    # tail: 4 -> 2 -> 1 (last fold writes the y chunk)
                    nc.vector.tensor_max(acc[:, 0], acc[:, 0], acc[:, 1])
                    ysl = y_sb[:, j * BCH : (j + 1) * BCH]
                    nc.vector.tensor_max(ysl, acc[:, 0, 0, :], acc[:, 0, 1, :])
                    if affine:
                        nc.vector.tensor_scalar(
                            out=ysl,
                            in0=ysl,
                            scalar1=sc[:],
                            scalar2=bi[:],
                            op0=mybir.AluOpType.mult,
                            op1=mybir.AluOpType.add,
                        )
                    nc.sync.dma_start(
                        out=y_d[:, j * BCH : (j + 1) * BCH], in_=ysl
                    )

    if fixup:
        split_multiwaits(nc)
    return nc


_CACHED_NC = None


def _get_nc():
    global _CACHED_NC
    if _CACHED_NC is None:
        _CACHED_NC = build_nc()
    return _CACHED_NC


def make_in_maps(x, ww, scale, bias):
    x = np.asarray(x)
    ww = np.asarray(ww)
    scale = np.asarray(scale)
    bias = np.asarray(bias)

    xf = np.ascontiguousarray(x.reshape(B, IDIM).T).astype(np.float32)  # (64, 2048)
    wwf = ww.reshape(FDIM, P, IDIM)
    sc = scale.reshape(FDIM).astype(np.float32)
    bi = bias.reshape(FDIM).astype(np.float32)

    in_maps = []
    for k in range(N_CORES):
        wk = wwf[k * F_SH : (k + 1) * F_SH]  # (128, 64, 64) = (f, p, i)
        wt = np.ascontiguousarray(wk.transpose(2, 1, 0)).astype(np.float32)  # (i,p,f)
        in_maps.append(
            {
                "xt": xf,
                "wt": wt,
                "scale": np.ascontiguousarray(
                    sc[k * F_SH : (k + 1) * F_SH].reshape(F_SH, 1)
                ),
                "bias": np.ascontiguousarray(
                    bi[k * F_SH : (k + 1) * F_SH].reshape(F_SH, 1)
                ),
            }
        )
    return in_maps


def kernel(x, ww, scale, bias):
    in_maps = make_in_maps(x, ww, scale, bias)
    trivial_affine = bool(
        np.all(np.asarray(scale) == 1.0) and np.all(np.asarray(bias) == 0.0)
    )
    nc = build_nc(affine=not trivial_affine)
    res = run_bass_kernel_spmd(nc, in_maps, list(range(N_CORES)))
    out = np.empty((FDIM, B), dtype=np.float32)
    for k in range(N_CORES):
        out[k * F_SH : (k + 1) * F_SH] = res.results[k]["y"].astype(np.float32)
    return np.ascontiguousarray(out.T)



# revision 6
# speedup vs baseline: 1.2803x; 1.1357x over previous
"""Trainium2 Bass kernel for nn_AffineLayer (topk_masking):
out[b, f] = max_p(x[b] . ww[f, p]) * scale[f] + bias[f]

Shapes (hardcoded per problem spec):
  x     (2048, 1, 8, 8)  -> xf (2048, 64)
  ww    (1024, 64, 1, 8, 8) -> wwf (1024, 64, 64)   (f, p, i)
  scale (1, 1024), bias (1, 1024)
  out   (2048, 1024)

Sharding: f tensor-parallel over 8 cores (f_shard = 128 per core), x replicated.

Per-core device layout (f on partitions):
  lhsT (stationary) = wT[:, p, :] : (i=64, f=128)  per p-plane
  rhs  (moving)     = xT[:, bchunk]: (i=64, b=512)
  psum out          = (f=128, b=512) per p-plane, 1 PSUM bank

The 64-way max over p is the bottleneck: every score must leave PSUM through
one of the only two PSUM-capable engines (DVE and ACT, both 1 elem/cycle/lane
for fp32). p-plane groups are split between:
  - DVE: running tensor_tensor(max) straight from PSUM into a 4-slot fp32 acc
  - ACT: activation(Copy) PSUM -> SBUF staging (cast to fp16: same 16-bit
    2x fold speed as bf16, 8x the mantissa precision), folded into 16-bit
    accumulators by DVE tensor_tensor at 2x packed rate.
Final per-chunk: fold acc slots, combine paths, apply scale/bias via one
tensor_scalar with per-partition (f) scalars, DMA out as (128f, 2048b).
Host reassembles and transposes to (2048, 1024).
"""

import os
import sys

if "/opt/trn_rl_repo" not in sys.path:
    sys.path.insert(0, "/opt/trn_rl_repo")

import numpy as np

import concourse.bass as bass
import concourse.mybir as mybir
from concourse.tile import TileContext
from concourse.bass_utils import run_bass_kernel_spmd

# Problem dims (hardcoded)
B, FDIM, P, IDIM = 2048, 1024, 64, 64
N_CORES = 8
F_SH = FDIM // N_CORES  # 128
BCH = 512  # b-chunk size (PSUM bank = 512 fp32)
NJ = B // BCH  # 4
GQ = int(os.environ.get("KGQ", "2"))  # p-planes per group (= PSUM banks)
NG = P // GQ  # groups
PSUM_BUFS = 8 // GQ

# ---- Tunables ----------------------------------------------------------
# Per-group drain assignment, length NG. "D" = DVE direct TT-max from PSUM;
# "V" = ACT copy -> staged, folded by DVE; "G" = ACT copy -> staged, folded
# by GPSIMD.
ASSIGN = os.environ.get("KASSIGN", "VVDVVVDVVVDVVVDVVVDVVVDVVVDVVVDV")
STAGE_BF16 = os.environ.get("KSTAGE_BF16", "1") == "1"
# Staging dtype: fp16 matches bf16's 2x DVE fold speed (both 16-bit) but has
# 10 mantissa bits vs 7 — scores (|s| < ~70) sit far inside fp16 range.
STAGE_DT_NAME = os.environ.get("KSTAGE_DT", "float16" if STAGE_BF16 else "float32")
# Matmul input dtype: float32r streams 1 row/cycle on the PE (vs 4 for fp32,
# which decomposes into 2 half-speed passes); same 4-byte layout as fp32.
MM_DT_NAME = os.environ.get("KMM_DT", "float32r")
# Unified accumulator: direct-drained groups also max into the bf16 staged
# acc (drops the separate fp32 acc + its tail folds; whole output ~bf16).
UNIFIED = os.environ.get("KUNIFIED", "0") == "1"
NWCH = int(os.environ.get("KNWCH", "16"))
REPS = int(os.environ.get("KREPS", "0"))  # >0: wrap body in a For_i repeat loop (bench only)
STAGE_BUFS = int(os.environ.get("KSTAGE_BUFS", "8"))
XT_CHUNKED = os.environ.get("KXT_CHUNKED", "1") == "1"
DQUAD = os.environ.get("KDQUAD", "0") == "1"  # D-groups drain as 4-bank quads
JINT = os.environ.get("KJINT", "0") == "1"  # interleave all b-chunks per p-position
# ------------------------------------------------------------------------

F32 = mybir.dt.float32
BF16 = mybir.dt.bfloat16
STAGE_DT = getattr(mybir.dt, STAGE_DT_NAME)
MM_DT = getattr(mybir.dt, MM_DT_NAME)
MX = mybir.AluOpType.max


def split_multiwaits(nc):
    """This walrus build allows at most ONE sem wait per instruction.
    Tile's wait assignment can emit several; hoist extras onto inserted
    sequencer nops immediately before the over-subscribed instruction
    (same engine, program order preserved => identical semantics)."""
    wid = 0
    for f in nc.m.functions:
        for bb in f.blocks:
            il = bb.instructions
            i = 0
            while i < len(il):
                ins = il[i]
                si = getattr(ins, "sync_info", None)
                if si is not None and si.on_wait and len(si.on_wait) > 1:
                    waits = list(si.on_wait)
                    si.on_wait = waits[-1:]
                    carriers = []
                    for w in waits[:-1]:
                        wid += 1
                        carriers.append(
                            mybir.InstNoOp(
                                name=f"WSPLIT-{wid}",
                                engine=ins.engine,
                                sync_info=mybir.SyncInfo(on_wait=[w], on_update=[]),
                            )
                        )
                    il[i:i] = carriers
                    i += len(carriers)
                i += 1


def build_nc_jint(assign=None, fixup=True, affine=True):
    """b-chunk-interleaved variant: iterate p-positions outer, all NJ b-chunks
    inner. Staged tiles hold one position x all chunks (NJ*GQ planes), folded
    by one DVE TT; accumulators span all chunks so the tails and the output
    DMA are whole-row ops."""
    assign = (assign or ASSIGN).split(";")[0]
    assert len(assign) in (16, NG) and set(assign) <= set("DV")
    if len(assign) != NG:
        assign = "".join(c * (NG // 16) for c in assign)
    last_d = assign.rfind("D")
    last_v = assign.rfind("V")

    nc = bass.Bass()
    xt_d = nc.dram_tensor("xt", [IDIM, B], MM_DT, kind="ExternalInput")
    wt_d = nc.dram_tensor("wt", [IDIM, P, F_SH], MM_DT, kind="ExternalInput")
    sc_d = nc.dram_tensor("scale", [F_SH, 1], F32, kind="ExternalInput")
    bi_d = nc.dram_tensor("bias", [F_SH, 1], F32, kind="ExternalInput")
    y_d = nc.dram_tensor("y", [F_SH, B], F32, kind="ExternalOutput")

    PW = P // NWCH

    with TileContext(nc) as tc:
        with (
            tc.tile_pool(name="const", bufs=1) as const,
            tc.tile_pool(name="psum", bufs=PSUM_BUFS, space="PSUM") as psum,
            tc.tile_pool(name="accs", bufs=2) as accs,
            tc.tile_pool(name="stage", bufs=STAGE_BUFS) as stage,
            tc.tile_pool(name="outs", bufs=2) as outs,
        ):
            xt = const.tile([IDIM, B], MM_DT)
            nc.sync.dma_start(out=xt[:, 0:BCH], in_=xt_d[:, 0:BCH])
            wchunks = [
                const.tile([IDIM, PW, F_SH], MM_DT, name=f"wt{c}") for c in range(NWCH)
            ]
            nc.sync.dma_start(out=wchunks[0][:], in_=wt_d[:, 0:PW, :])
            for c in range(1, NJ):
                nc.sync.dma_start(
                    out=xt[:, c * BCH : (c + 1) * BCH],
                    in_=xt_d[:, c * BCH : (c + 1) * BCH],
                )
            for c in range(1, NWCH):
                nc.sync.dma_start(
                    out=wchunks[c][:], in_=wt_d[:, c * PW : (c + 1) * PW, :]
                )
            sc = const.tile([F_SH, 1], F32)
            nc.sync.dma_start(out=sc[:], in_=sc_d[:])
            bi = const.tile([F_SH, 1], F32)
            nc.sync.dma_start(out=bi[:], in_=bi_d[:])
            warm = const.tile([F_SH, 2], F32)
            nc.vector.memset(warm[:], 0.0)
            nc.scalar.activation(
                out=warm[:, 1:2], in_=warm[:, 0:1],
                func=mybir.ActivationFunctionType.Copy,
            )

            import contextlib

            loop_cm = (
                tc.For_i(0, REPS, 1, hint_engines=(mybir.EngineType.PE,))
                if REPS > 0
                else contextlib.nullcontext()
            )
            with loop_cm:
                acc_d = accs.tile([F_SH, NJ, GQ, BCH], F32, tag="acc_d")
                acc_v = accs.tile([F_SH, NJ, GQ, BCH], STAGE_DT, tag="acc_v")
                n_d = n_v = 0

                def fold_gq(acc):
                    w = GQ
                    while w > 1:
                        h = w // 2
                        nc.vector.tensor_max(
                            acc[:, :, 0:h, :], acc[:, :, 0:h, :], acc[:, :, h:w, :]
                        )
                        w = h

                for g in range(NG):
                    if assign[g] == "D":
                        for j in range(NJ):
                            pt = psum.tile([F_SH, GQ, BCH], F32, tag="ps")
                            for q in range(GQ):
                                p = GQ * g + q
                                nc.tensor.matmul(
                                    pt[:, q, :],
                                    wchunks[p // PW][:, p % PW, :],
                                    xt[:, j * BCH : (j + 1) * BCH],
                                    start=True,
                                    stop=True,
                                )
                            dst = acc_d[:, j]
                            if n_d == 0:
                                nc.vector.tensor_copy(out=dst, in_=pt[:])
                            else:
                                nc.vector.tensor_max(dst, pt[:], dst)
                        n_d += 1
                        if g == last_d and last_d > last_v:
                            fold_gq(acc_d)
                    else:
                        st = stage.tile([F_SH, NJ, GQ, BCH], STAGE_DT, tag="st")
                        for j in range(NJ):
                            pt = psum.tile([F_SH, GQ, BCH], F32, tag="ps")
                            for q in range(GQ):
                                p = GQ * g + q
                                nc.tensor.matmul(
                                    pt[:, q, :],
                                    wchunks[p // PW][:, p % PW, :],
                                    xt[:, j * BCH : (j + 1) * BCH],
                                    start=True,
                                    stop=True,
                                )
                            nc.scalar.activation(
                                out=st[:, j],
                                in_=pt[:],
                                func=mybir.ActivationFunctionType.Copy,
                            )
                        if n_v == 0:
                            nc.vector.tensor_copy(out=acc_v[:], in_=st[:])
                        else:
                            nc.vector.tensor_max(acc_v[:], st[:], acc_v[:])
                        n_v += 1
                        if g == last_v and last_v > last_d:
                            fold_gq(acc_v)

                # ---- tails: whole-row ops across all chunks ------------
                if n_v and last_v < last_d:
                    fold_gq(acc_v)
                if n_d and last_d < last_v:
                    fold_gq(acc_d)
                staged = acc_v[:, :, 0, :] if n_v else None  # (F_SH, NJ, BCH)
                direct = acc_d[:, :, 0, :] if n_d else None
                outt = outs.tile([F_SH, NJ, BCH], F32, tag="outt")
                if direct is not None and staged is not None:
                    nc.vector.tensor_max(outt[:], direct, staged)
                    src = outt[:]
                elif direct is not None:
                    src = direct
                else:
                    src = staged
                if affine:
                    nc.vector.tensor_scalar(
                        out=outt[:],
                        in0=src,
                        scalar1=sc[:],
                        scalar2=bi[:],
                        op0=mybir.AluOpType.mult,
                        op1=mybir.AluOpType.add,
                    )
                    src = outt[:]
                elif src is not outt[:] and src.dtype != F32:
                    nc.vector.tensor_copy(out=outt[:], in_=src)
                    src = outt[:]
                nc.sync.dma_start(out=y_d[:], in_=src)

    if fixup:
        split_multiwaits(nc)
    return nc



def build_nc(assign=None, fixup=True, affine=True):
    kv = os.environ.get("KV", "2")
    if kv == "3":
        return build_nc_v3(fixup=fixup, affine=affine)
    if kv == "2":
        return build_nc_v2(fixup=fixup, affine=affine)
    if JINT:
        return build_nc_jint(assign=assign, fixup=fixup, affine=affine)
    assign = assign or ASSIGN
    pats = assign.split(";")
    if len(pats) == 1:
        pats = pats * NJ
    assert len(pats) == NJ
    expanded = []
    for p_ in pats:
        assert len(p_) in (16, NG) and set(p_) <= set("DV")
        if len(p_) != NG:
            p_ = "".join(c * (NG // 16) for c in p_)
        expanded.append(p_)
    pats = expanded

    nc = bass.Bass()
    xt_d = nc.dram_tensor("xt", [IDIM, B], MM_DT, kind="ExternalInput")
    wt_d = nc.dram_tensor("wt", [IDIM, P, F_SH], MM_DT, kind="ExternalInput")
    sc_d = nc.dram_tensor("scale", [F_SH, 1], F32, kind="ExternalInput")
    bi_d = nc.dram_tensor("bias", [F_SH, 1], F32, kind="ExternalInput")
    y_d = nc.dram_tensor("y", [F_SH, B], F32, kind="ExternalOutput")

    PW = P // NWCH  # p-planes per weight chunk
    VS = 2 * GQ  # staged-pair slot count (2 groups per staged tile)

    with TileContext(nc) as tc:
        with (
            tc.tile_pool(name="const", bufs=1) as const,
            tc.tile_pool(name="psum", bufs=PSUM_BUFS, space="PSUM") as psum,
            tc.tile_pool(
                name="accs", bufs=int(os.environ.get("KACC_BUFS", "3"))
            ) as accs,
            tc.tile_pool(name="stage", bufs=STAGE_BUFS) as stage,
            tc.tile_pool(
                name="outs", bufs=int(os.environ.get("KOUT_BUFS", "3"))
            ) as outs,
        ):
            # input loads: first-needed chunks first so group 0 starts ASAP
            xt = const.tile([IDIM, B], MM_DT)
            wchunks = [
                const.tile([IDIM, PW, F_SH], MM_DT, name=f"wt{c}") for c in range(NWCH)
            ]
            nc.sync.dma_start(out=xt[:, 0:BCH], in_=xt_d[:, 0:BCH])
            nc.sync.dma_start(out=wchunks[0][:], in_=wt_d[:, 0:PW, :])
            nc.sync.dma_start(out=wchunks[1][:], in_=wt_d[:, PW : 2 * PW, :])
            for c in range(2, NWCH):
                nc.sync.dma_start(
                    out=wchunks[c][:], in_=wt_d[:, c * PW : (c + 1) * PW, :]
                )
            for c in range(1, NJ):
                nc.sync.dma_start(
                    out=xt[:, c * BCH : (c + 1) * BCH],
                    in_=xt_d[:, c * BCH : (c + 1) * BCH],
                )
            sc = const.tile([F_SH, 1], F32)
            nc.sync.dma_start(out=sc[:], in_=sc_d[:])
            bi = const.tile([F_SH, 1], F32)
            nc.sync.dma_start(out=bi[:], in_=bi_d[:])
            warm = const.tile([F_SH, 2], F32)
            nc.vector.memset(warm[:], 0.0)
            nc.scalar.activation(
                out=warm[:, 1:2], in_=warm[:, 0:1],
                func=mybir.ActivationFunctionType.Copy,
            )

            import contextlib

            loop_cm = (
                tc.For_i(0, REPS, 1, hint_engines=(mybir.EngineType.PE,))
                if REPS > 0
                else contextlib.nullcontext()
            )
            with loop_cm:
              for j in range(NJ):
                assign_j = pats[j]
                last_d = assign_j.rfind("D")
                rhs = xt[:, j * BCH : (j + 1) * BCH]
                DS = 4 if DQUAD else GQ
                acc_d = accs.tile([F_SH, DS, BCH], F32, tag="acc_d")
                acc_v = accs.tile([F_SH, VS, BCH], STAGE_DT, tag="acc_v")
                n_d = n_v = 0
                half = 0  # staged-pair fill state
                st = None

                def flush_pair(full):
                    nonlocal n_v, st
                    if full:
                        src = st[:].rearrange("p a g b -> p (a g) b")
                        dst = acc_v[:]
                    else:
                        src = st[:, 0]
                        dst = acc_v[:, 0:GQ, :]
                    if n_v == 0:
                        nc.vector.tensor_copy(out=dst, in_=src)
                    else:
                        nc.vector.tensor_max(dst, src, dst)
                    n_v += 1
                    st = None

                # build token schedule: D-pairs become 4-bank quads in DQUAD mode
                tokens = []
                g = 0
                while g < NG:
                    if (
                        DQUAD
                        and assign_j[g] == "D"
                    ):
                        assert g + 1 < NG and assign_j[g + 1] == "D", (
                            "KDQUAD=1 requires D groups in adjacent pairs"
                        )
                        tokens.append(("D", g, 2 * GQ))
                        g += 2
                    else:
                        tokens.append((assign_j[g], g, GQ))
                        g += 1
                n_dtok = sum(1 for t in tokens if t[0] == "D")
                dtok_i = 0
                for kind, g0, nplanes in tokens:
                    if kind == "D" and DQUAD:
                        pt = psum.tile([F_SH, 4, BCH], F32, tag="psd", bufs=1, name="ptd")
                    else:
                        pt = psum.tile(
                            [F_SH, GQ, BCH],
                            F32,
                            tag="ps",
                            bufs=2 if DQUAD else PSUM_BUFS,
                            name="ptv",
                        )
                    for q in range(nplanes):
                        p = GQ * g0 + q
                        nc.tensor.matmul(
                            pt[:, q, :],
                            wchunks[p // PW][:, p % PW, :],
                            rhs,
                            start=True,
                            stop=True,
                        )
                    if kind == "D":
                        dst = acc_d[:] if nplanes == DS else acc_d[:, 0:nplanes, :]
                        if n_d == 0:
                            assert nplanes == DS, "first D token must fill acc_d"
                            nc.vector.tensor_copy(out=dst, in_=pt[:])
                        else:
                            nc.vector.tensor_max(dst, pt[:], dst)
                        n_d += 1
                        dtok_i += 1
                        if dtok_i == n_dtok:
                            w = DS
                            while w > 1:
                                h = w // 2
                                nc.vector.tensor_max(
                                    acc_d[:, 0:h, :],
                                    acc_d[:, 0:h, :],
                                    acc_d[:, h:w, :],
                                )
                                w = h
                    else:
                        if st is None:
                            st = stage.tile([F_SH, 2, GQ, BCH], STAGE_DT, tag="st")
                        nc.scalar.activation(
                            out=st[:, half],
                            in_=pt[:],
                            func=mybir.ActivationFunctionType.Copy,
                        )
                        half ^= 1
                        if half == 0:
                            flush_pair(full=True)
                if half == 1:
                    flush_pair(full=False)

                # ---- tails ------------------------------------------------
                staged = None
                if n_v:
                    w = VS
                    while w > 1:
                        h = w // 2
                        nc.vector.tensor_max(
                            acc_v[:, 0:h, :], acc_v[:, 0:h, :], acc_v[:, h:w, :]
                        )
                        w = h
                    staged = acc_v[:, 0, :]
                direct = acc_d[:, 0, :] if n_d else None

                outt = outs.tile([F_SH, BCH], F32, tag="outt")
                if direct is not None and staged is not None:
                    nc.vector.tensor_max(outt[:], direct, staged)  # mixed dtype OK
                    src = outt[:]
                elif direct is not None:
                    src = direct
                else:
                    src = staged
                if affine:
                    nc.vector.tensor_scalar(
                        out=outt[:],
                        in0=src,
                        scalar1=sc[:],
                        scalar2=bi[:],
                        op0=mybir.AluOpType.mult,
                        op1=mybir.AluOpType.add,
                    )
                    src = outt[:]
                elif src is not outt[:] and src.dtype != F32:
                    nc.vector.tensor_copy(out=outt[:], in_=src)
                    src = outt[:]
                nc.sync.dma_start(out=y_d[:, j * BCH : (j + 1) * BCH], in_=src)

    if fixup:
        split_multiwaits(nc)
    return nc


def build_nc_v3(assign=None, fixup=True, affine=True):
    """v3: like v2 but PSUM tiles are 4 banks (bufs=2) so each ACT copy
    moves 4 planes in one instruction (481ns/plane vs 526). Pattern is
    over 16 4-plane groups per chunk; ';'-separated per-chunk patterns
    alternate to hit fractional V:D ratios."""
    pats = (assign or os.environ.get("KPAT16", "VDVVVVVVDVVVVVVD;VVDVVVVVVDVVVVVV")).split(";")
    if len(pats) == 1:
        pats = pats * NJ
    elif len(pats) == 2:
        pats = [pats[0], pats[1]] * (NJ // 2)
    assert len(pats) == NJ
    for p_ in pats:
        assert len(p_) == 16 and set(p_) <= set("DV") and p_[0] == "V"

    nc = bass.Bass()
    xt_d = nc.dram_tensor("xt", [IDIM, B], MM_DT, kind="ExternalInput")
    wt_d = nc.dram_tensor("wt", [IDIM, P, F_SH], MM_DT, kind="ExternalInput")
    sc_d = nc.dram_tensor("scale", [F_SH, 1], F32, kind="ExternalInput")
    bi_d = nc.dram_tensor("bias", [F_SH, 1], F32, kind="ExternalInput")
    y_d = nc.dram_tensor("y", [F_SH, B], STAGE_DT, kind="ExternalOutput")

    PW = P // NWCH

    with TileContext(nc) as tc:
        with (
            tc.tile_pool(name="const", bufs=1) as const,
            tc.tile_pool(name="psum", bufs=2, space="PSUM") as psum,
            tc.tile_pool(name="accs", bufs=2) as accs,
            tc.tile_pool(name="stage", bufs=int(os.environ.get("KSTB", "4"))) as stage,
        ):
            xt = const.tile([IDIM, B], MM_DT)
            wchunks = [
                const.tile([IDIM, PW, F_SH], MM_DT, name=f"wt{c}") for c in range(NWCH)
            ]
            nc.sync.dma_start(out=xt[:, 0:BCH], in_=xt_d[:, 0:BCH])
            nc.sync.dma_start(out=wchunks[0][:], in_=wt_d[:, 0:PW, :])
            nc.sync.dma_start(out=wchunks[1][:], in_=wt_d[:, PW : 2 * PW, :])
            for c in range(2, NWCH):
                nc.sync.dma_start(
                    out=wchunks[c][:], in_=wt_d[:, c * PW : (c + 1) * PW, :]
                )
            for c in range(1, NJ):
                nc.sync.dma_start(
                    out=xt[:, c * BCH : (c + 1) * BCH],
                    in_=xt_d[:, c * BCH : (c + 1) * BCH],
                )
            sc = const.tile([F_SH, 1], F32)
            nc.sync.dma_start(out=sc[:], in_=sc_d[:])
            bi = const.tile([F_SH, 1], F32)
            nc.sync.dma_start(out=bi[:], in_=bi_d[:])
            y_sb = const.tile([F_SH, B], STAGE_DT, name="ysb")
            warm = const.tile([F_SH, 2], F32)
            nc.vector.memset(warm[:], 0.0)
            nc.scalar.activation(
                out=warm[:, 1:2], in_=warm[:, 0:1],
                func=mybir.ActivationFunctionType.Copy,
            )

            import contextlib

            loop_cm = (
                tc.For_i(0, REPS, 1, hint_engines=(mybir.EngineType.PE,))
                if REPS > 0
                else contextlib.nullcontext()
            )
            with loop_cm:
                for j in range(NJ):
                    pat = pats[j]
                    rhs = xt[:, j * BCH : (j + 1) * BCH]
                    acc = accs.tile([F_SH, 4, BCH], STAGE_DT, tag="acc")
                    filled = False

                    for g in range(16):
                        pt = psum.tile([F_SH, 4, BCH], F32, tag="ps")
                        for q in range(4):
                            p = 4 * g + q
                            nc.tensor.matmul(
                                pt[:, q, :],
                                wchunks[p // PW][:, p % PW, :],
                                rhs,
                                start=True,
                                stop=True,
                            )
                        if pat[g] == "V":
                            if not filled:
                                nc.scalar.activation(
                                    out=acc[:],
                                    in_=pt[:],
                                    func=mybir.ActivationFunctionType.Copy,
                                )
                                filled = True
                            else:
                                st = stage.tile([F_SH, 4, BCH], STAGE_DT, tag="st")
                                nc.scalar.activation(
                                    out=st[:],
                                    in_=pt[:],
                                    func=mybir.ActivationFunctionType.Copy,
                                )
                                nc.vector.tensor_max(acc[:], st[:], acc[:])
                        else:
                            nc.vector.tensor_max(acc[:], pt[:], acc[:])

                    # tail: 4 -> 2 -> 1 (last fold writes the y chunk)
                    nc.vector.tensor_max(
                        acc[:, 0:2, :], acc[:, 0:2, :], acc[:, 2:4, :]
                    )
                    ysl = y_sb[:, j * BCH : (j + 1) * BCH]
                    nc.vector.tensor_max(ysl, acc[:, 0, :], acc[:, 1, :])
                    if affine:
                        nc.vector.tensor_scalar(
                            out=ysl,
                            in0=ysl,
                            scalar1=sc[:],
                            scalar2=bi[:],
                            op0=mybir.AluOpType.mult,
                            op1=mybir.AluOpType.add,
                        )
                    nc.sync.dma_start(
                        out=y_d[:, j * BCH : (j + 1) * BCH], in_=ysl
                    )

    if fixup:
        split_multiwaits(nc)
    return nc


def build_nc_v2(assign=None, fixup=True, affine=True):
    """v2: unified fp16 accumulator drain.

    Per b-chunk (BCH=512), 32 GQ=2 PSUM groups (4 in flight):
      - V-groups: ACT copies PSUM -> fp16. The first two fill the 4-slot
        acc directly (no TensorCopy init); later ones pair up in staged
        tiles folded into acc by DVE fp16 TTs at 2x.
      - D-groups: DVE tensor_tensor(max) PSUM(fp32) x acc(fp16) -> acc.
    Tail: two fp16 folds (4->2->1) with the last writing the y chunk;
    y is fp16 in DRAM, host upcasts. Affine (when non-trivial) is a 4x
    fp16 tensor_scalar on the y chunk.
    """
    assign = assign or os.environ.get(
        "KPAT", "VVDVVDVVDVVDVVDVVDVVDVVDVVDVVVVV"
    )
    assert len(assign) == 32 and set(assign) <= set("DV")
    assert assign[0] == "V" and assign[1] == "V", "first two groups must fill acc"

    nc = bass.Bass()
    xt_d = nc.dram_tensor("xt", [IDIM, B], MM_DT, kind="ExternalInput")
    wt_d = nc.dram_tensor("wt", [IDIM, P, F_SH], MM_DT, kind="ExternalInput")
    sc_d = nc.dram_tensor("scale", [F_SH, 1], F32, kind="ExternalInput")
    bi_d = nc.dram_tensor("bias", [F_SH, 1], F32, kind="ExternalInput")
    y_d = nc.dram_tensor("y", [F_SH, B], STAGE_DT, kind="ExternalOutput")

    PW = P // NWCH

    with TileContext(nc) as tc:
        with (
            tc.tile_pool(name="const", bufs=1) as const,
            tc.tile_pool(name="psum", bufs=4, space="PSUM") as psum,
            tc.tile_pool(name="accs", bufs=2) as accs,
            tc.tile_pool(name="stage", bufs=int(os.environ.get("KSTB", "6"))) as stage,
        ):
            xt = const.tile([IDIM, B], MM_DT)
            wchunks = [
                const.tile([IDIM, PW, F_SH], MM_DT, name=f"wt{c}") for c in range(NWCH)
            ]
            nc.sync.dma_start(out=xt[:, 0:BCH], in_=xt_d[:, 0:BCH])
            nc.sync.dma_start(out=wchunks[0][:], in_=wt_d[:, 0:PW, :])
            nc.sync.dma_start(out=wchunks[1][:], in_=wt_d[:, PW : 2 * PW, :])
            for c in range(2, NWCH):
                nc.sync.dma_start(
                    out=wchunks[c][:], in_=wt_d[:, c * PW : (c + 1) * PW, :]
                )
            for c in range(1, NJ):
                nc.sync.dma_start(
                    out=xt[:, c * BCH : (c + 1) * BCH],
                    in_=xt_d[:, c * BCH : (c + 1) * BCH],
                )
            sc = const.tile([F_SH, 1], F32)
            nc.sync.dma_start(out=sc[:], in_=sc_d[:])
            bi = const.tile([F_SH, 1], F32)
            nc.sync.dma_start(out=bi[:], in_=bi_d[:])
            y_sb = const.tile([F_SH, B], STAGE_DT, name="ysb")
            warm = const.tile([F_SH, 2], F32)
            nc.vector.memset(warm[:], 0.0)
            nc.scalar.activation(
                out=warm[:, 1:2], in_=warm[:, 0:1],
                func=mybir.ActivationFunctionType.Copy,
            )

            import contextlib

            loop_cm = (
                tc.For_i(0, REPS, 1, hint_engines=(mybir.EngineType.PE,))
                if REPS > 0
                else contextlib.nullcontext()
            )
            with loop_cm:
                for j in range(NJ):
                    rhs = xt[:, j * BCH : (j + 1) * BCH]
                    acc = accs.tile([F_SH, 2, GQ, BCH], STAGE_DT, tag="acc")
                    acc4 = acc[:].rearrange("p a g b -> p (a g) b")
                    nfill = 0  # V-groups that went straight into acc
                    pend = None  # half-filled staged pair tile
                    st = None

                    def mm_group(g):
                        pt = psum.tile([F_SH, GQ, BCH], F32, tag="ps")
                        for q in range(GQ):
                            p = GQ * g + q
                            nc.tensor.matmul(
                                pt[:, q, :],
                                wchunks[p // PW][:, p % PW, :],
                                rhs,
                                start=True,
                                stop=True,
                            )
                        return pt

                    for g in range(NG):
                        pt = mm_group(g)
                        if assign[g] == "V":
                            if nfill < 2:
                                nc.scalar.activation(
                                    out=acc[:, nfill],
                                    in_=pt[:],
                                    func=mybir.ActivationFunctionType.Copy,
                                )
                                nfill += 1
                            else:
                                if st is None:
                                    st = stage.tile(
                                        [F_SH, 2, GQ, BCH], STAGE_DT, tag="st"
                                    )
                                    half = 0
                                else:
                                    half = 1
                                nc.scalar.activation(
                                    out=st[:, half],
                                    in_=pt[:],
                                    func=mybir.ActivationFunctionType.Copy,
                                )
                                if half == 1:
                                    nc.vector.tensor_max(
                                        acc4,
                                        st[:].rearrange("p a g b -> p (a g) b"),
                                        acc4,
                                    )
                                    st = None
                        else:
                            # direct: PSUM fp32 x acc fp16 -> acc fp16 (1x)
                            nc.vector.tensor_max(acc[:, 0], pt[:], acc[:, 0])
                    if st is not None:  # odd V-group left staged
                        nc.vector.tensor_max(acc[:, 1], st[:, 0], acc[:, 1])
                        st = None

                    # tail: 4 -> 2 -> 1 (last fold writes the y chunk)
                    nc.vector.tensor_max(acc[:, 0], acc[:, 0], acc[:, 1])
                    ysl = y_sb[:, j * BCH : (j + 1) * BCH]
                    nc.vector.tensor_max(ysl, acc[:, 0, 0, :], acc[:, 0, 1, :])
                    if affine:
                        nc.vector.tensor_scalar(
                            out=ysl,
                            in0=ysl,
                            scalar1=sc[:],
                            scalar2=bi[:],
                            op0=mybir.AluOpType.mult,
                            op1=mybir.AluOpType.add,
                        )
                    nc.sync.dma_start(
                        out=y_d[:, j * BCH : (j + 1) * BCH], in_=ysl
                    )

    if fixup:
        split_multiwaits(nc)
    return nc


_CACHED_NC = None


def _get_nc():
    global _CACHED_NC
    if _CACHED_NC is None:
        _CACHED_NC = build_nc()
    return _CACHED_NC


def make_in_maps(x, ww, scale, bias):
    x = np.asarray(x)
    ww = np.asarray(ww)
    scale = np.asarray(scale)
    bias = np.asarray(bias)

    xf = np.ascontiguousarray(x.reshape(B, IDIM).T).astype(np.float32)  # (64, 2048)
    wwf = ww.reshape(FDIM, P, IDIM)
    sc = scale.reshape(FDIM).astype(np.float32)
    bi = bias.reshape(FDIM).astype(np.float32)

    in_maps = []
    for k in range(N_CORES):
        wk = wwf[k * F_SH : (k + 1) * F_SH]  # (128, 64, 64) = (f, p, i)
        wt = np.ascontiguousarray(wk.transpose(2, 1, 0)).astype(np.float32)  # (i,p,f)
        in_maps.append(
            {
                "xt": xf,
                "wt": wt,
                "scale": np.ascontiguousarray(
                    sc[k * F_SH : (k + 1) * F_SH].reshape(F_SH, 1)
                ),
                "bias": np.ascontiguousarray(
                    bi[k * F_SH : (k + 1) * F_SH].reshape(F_SH, 1)
                ),
            }
        )
    return in_maps


def kernel(x, ww, scale, bias):
    in_maps = make_in_maps(x, ww, scale, bias)
    trivial_affine = bool(
        np.all(np.asarray(scale) == 1.0) and np.all(np.asarray(bias) == 0.0)
    )
    nc = build_nc(affine=not trivial_affine)
    res = run_bass_kernel_spmd(nc, in_maps, list(range(N_CORES)))
    out = np.empty((FDIM, B), dtype=np.float32)
    for k in range(N_CORES):
        out[k * F_SH : (k + 1) * F_SH] = res.results[k]["y"].astype(np.float32)
    return np.ascontiguousarray(out.T)

